# revision 18
# baseline (speedup 1.0000x reference)
"""Trainium2 Bass kernel for nn_Loss_20933670601009 (gathered-prob NLL loss).

The loss touches 3 elements per (l, b) position: one gathered prob from each
of the rule/token/reference tables. Instead of streaming ~566MB of prob
tensors, each core fetches exactly the values it needs:

  - rule + token values (8 x 128 = 1024 per core): indirect-DMA element
    gathers straight from HBM. HW consumes ONE offset per partition row per
    instruction, so 8 instructions of [P,1] cover 2 components x 4
    positions-per-partition. All index arithmetic, validity (gt == -1) and
    mask handling is precomputed on the host into the offsets via two
    sentinel elements appended to the flat buffer (flat[N]=0.0, flat[N+1]=1.0;
    masked positions read prob 1.0 so ln(1)=0 drops out). The offsets ride in
    a minimal [P,8] first DMA so the gather train starts as early as possible.
  - reference values: the whole per-core reference table is only 1MB, so it
    is DMA'd into SBUF (overlapping the gather train) in an ap_gather-ready,
    host-pre-permuted layout; ONE gpsimd ap_gather instruction (~0.4us)
    extracts the per-position elements. ap_gather applies each index slot to
    all 16 partitions of a group, so only slots whose partition matches the
    position's home row carry real values; a host-shipped {0,1} mask and a
    16-wide strided reduce select them. The ap_gather ucode library is loaded
    as the FIRST gpsimd instruction so its load overlaps the prologue +
    gather train (the indirect-DMA ucode is resident and unaffected).

Position mapping (per core, NPOS=512): position q lives at partition
p = 16*(q//64) + q%16, slot j = (q%64)//16, which makes one ap_gather
(64 idx slots per 16-partition group) cover all 512 positions.

prob + (prob<eps)*eps is replaced by max(prob, eps): identical unless
0 < prob < 1e-7, which for sums of uniform(0,1) draws is a ~1e-21 event.

Sharding: data-parallel over L_a (128 rows -> 16 rows x 8 cores, 512
positions per core). Per-core partials are summed on the host; the on-device
-1/32 weight reproduces mean-over-batch of per-sequence sums.
"""

import os
import sys

import numpy as np

for _p in ("/opt/trn_rl_repo", "/root/.axon_site/_ro/trn_rl_repo"):
    if os.path.isdir(_p) and _p not in sys.path:
        sys.path.insert(0, _p)

L_A, B = 128, 32
V_RULE, V_TOK, V_REF = 2048, 32000, 512
EPS = 1e-07
N_CORES = 8
L_SH = L_A // N_CORES            # 16 sequence rows per core
NPOS = L_SH * B                  # 512 positions per core
P = 128                          # SBUF partitions
J = NPOS // P                    # 4 positions per partition
N_FLAT = NPOS * (V_RULE + V_TOK)  # rule || token flat buffer
ZERO_IDX = N_FLAT                # sentinel: flat[N_FLAT] = 0.0
ONE_IDX = N_FLAT + 1             # sentinel: flat[N_FLAT+1] = 1.0

# aux (int32 [P, 67]):
#   col  0      f32 bits of -1/B (matmul weight)
#   cols 1:65   f32 bits of the ap_gather select mask [P, 64]
#   cols 65:67  int16 pairs: ap_gather indices [P, 4]
AUX_W = 67

_CACHE = {}


def _build():
    """Build + compile the per-core Bass module (same NEFF on all 8 cores)."""
    import concourse.bacc as bacc
    import concourse.bass as bass
    import concourse.mybir as mybir
    import concourse.tile as tile
    from concourse import library_config

    f32 = mybir.dt.float32
    i16 = mybir.dt.int16
    i32 = mybir.dt.int32
    alu = mybir.AluOpType

    nc = bacc.Bacc(
        "TRN2",
        target_bir_lowering=False,
        debug=False,
        enable_asserts=False,
        num_devices=N_CORES,
    )

    offs_d = nc.dram_tensor("offs", [P, 2 * J], i32, kind="ExternalInput").ap()
    aux_d = nc.dram_tensor("aux", [P, AUX_W], i32, kind="ExternalInput").ap()
    flat_d = nc.dram_tensor(
        "probs_flat", [N_FLAT + 2, 1], f32, kind="ExternalInput"
    ).ap()
    ref_d = nc.dram_tensor("ref_shuf", [P, J * V_REF], f32, kind="ExternalInput").ap()
    out_d = nc.dram_tensor("out", [J, 1], f32, kind="ExternalOutput").ap()

    with tile.TileContext(nc) as tc:
        with (
            tc.tile_pool(name="sb", bufs=1) as pool,
            tc.tile_pool(name="ps", bufs=1, space="PSUM") as psum,
        ):
            # ucode load; overlaps the prologue + gather train below
            nc.gpsimd.load_library(library_config.ap_gather)

            offs = pool.tile([P, 2 * J], i32)
            nc.sync.dma_start(out=offs[:], in_=offs_d[:])
            reftbl = pool.tile([P, J * V_REF], f32)
            nc.sync.dma_start(out=reftbl[:], in_=ref_d[:])
            aux = pool.tile([P, AUX_W], i32)
            nc.sync.dma_start(out=aux[:], in_=aux_d[:])
            negw = aux[:, 0:1].bitcast(f32)
            val01 = aux[:, 1:65].bitcast(f32)
            apgidx = aux[:, 65:67].bitcast(i16)

            # rule + token element gathers (sentinel-encoded offsets)
            gv = pool.tile([P, 2 * J], f32)
            for col in range(2 * J):
                nc.gpsimd.indirect_dma_start(
                    out=gv[:, col:col + 1],
                    out_offset=None,
                    in_=flat_d[:],
                    in_offset=bass.IndirectOffsetOnAxis(
                        ap=offs[:, col:col + 1], axis=0
                    ),
                )

            # reference values: one ap_gather + select-mask + 16-wide reduce
            aout = pool.tile([P, 64], f32)
            nc.gpsimd.ap_gather(
                out_ap=aout[:], in_ap=reftbl[:], idxs_ap=apgidx,
                channels=P, num_elems=J * V_REF, d=1, num_idxs=64,
            )
            # s[p, j] = rule + token: issued FIRST on the DVE queue since it
            # only depends on the gather train, not the (later) ap_gather
            s = pool.tile([P, J], f32)
            nc.vector.reduce_sum(
                out=s[:],
                in_=gv[:].rearrange("p (j c) -> p j c", c=2),
                axis=mybir.AxisListType.X,
            )
            am = pool.tile([P, 64], f32)
            nc.vector.tensor_mul(out=am[:], in0=aout[:], in1=val01)
            refs = pool.tile([P, J], f32)
            nc.vector.reduce_sum(
                out=refs[:],
                in_=am[:].rearrange("p (j i) -> p j i", i=16),
                axis=mybir.AxisListType.X,
            )
            nc.vector.tensor_add(out=s[:], in0=s[:], in1=refs[:])
            # max(prob, eps) ~ prob + (prob < eps) * eps (see module docstring)
            nc.vector.tensor_scalar(
                out=s[:], in0=s[:], scalar1=EPS, scalar2=None, op0=alu.max
            )

            ln = pool.tile([P, J], f32)
            nc.scalar.activation(
                out=ln[:], in_=s[:], func=mybir.ActivationFunctionType.Ln
            )

            # partition reduction via PE; weight -1/B folds negation + mean.
            # The matmul contracts partitions directly from ln [P, J], so the
            # free-axis reduce moves to the host (sums 4 values per core).
            acc = psum.tile([J, 1], f32)
            nc.tensor.matmul(out=acc[:], lhsT=ln[:], rhs=negw, start=True, stop=True)
            res = pool.tile([J, 1], f32)
            nc.scalar.copy(out=res[:], in_=acc[:])
            nc.sync.dma_start(out=out_d[:], in_=res[:])

    nc.compile()
    return nc


def get_nc():
    if "nc" not in _CACHE:
        _CACHE["nc"] = _build()
    return _CACHE["nc"]


# position q -> (partition, slot): p = 16*(q//64) + q%16, j = (q%64)//16
_Q = np.arange(NPOS, dtype=np.int64)
_QP = 16 * (_Q // 64) + _Q % 16
_QJ = (_Q % 64) // 16
# qmap[p, j] = q
_QMAP = np.empty((P, J), np.int64)
_QMAP[_QP, _QJ] = _Q


def make_in_maps(rule_probs, token_probs, reference_probs, ground_truth_actions, mask):
    """Shard the full inputs into 8 per-core input maps."""
    rule_probs = np.ascontiguousarray(np.asarray(rule_probs, dtype=np.float32))
    token_probs = np.ascontiguousarray(np.asarray(token_probs, dtype=np.float32))
    reference_probs = np.ascontiguousarray(np.asarray(reference_probs, dtype=np.float32))
    gt = np.asarray(ground_truth_actions, dtype=np.int32)
    mask = np.asarray(mask, dtype=np.int32)

    negw_bits = np.float32(-1.0 / B).view(np.int32)
    gi = np.arange(64, dtype=np.int64)
    p_idx = np.arange(P)[:, None]
    q_of = 64 * (p_idx // 16) + gi[None, :]          # [P, 64]
    slot_live = (gi[None, :] % 16) == (p_idx % 16)   # [P, 64]

    in_maps = []
    for i in range(N_CORES):
        lo, hi = i * L_SH, (i + 1) * L_SH
        gt_sh = gt[lo:hi].reshape(NPOS, 3).astype(np.int64)
        m_sh = mask[lo:hi].reshape(NPOS)
        ref_sh = reference_probs[lo:hi].reshape(NPOS, V_REF)

        # rule/token offsets with sentinel encoding
        offs_t = np.zeros((P, 2 * J), np.int32)
        segs = (0, NPOS * V_RULE)
        vs = (V_RULE, V_TOK)
        for c in range(2):
            offs = segs[c] + _Q * vs[c] + np.clip(gt_sh[:, c], 0, None)
            offs[gt_sh[:, c] < 0] = ZERO_IDX
            offs[m_sh == 0] = ONE_IDX if c == 0 else ZERO_IDX
            offs_t[_QP, 2 * _QJ + c] = offs.astype(np.int32)

        aux = np.zeros((P, AUX_W), np.int32)
        aux[:, 0] = negw_bits
        # ap_gather select mask: slot i live on partition p iff i%16 == p%16
        # and the ref component of q = 64*(p//16)+i is valid & unmasked
        ref_ok = (gt_sh[:, 2] >= 0) & (m_sh == 1)
        val01 = np.zeros((P, 64), np.float32)
        val01[:] = slot_live & ref_ok[q_of]
        aux[:, 1:65] = val01.view(np.int32)
        # ap_gather indices: per group g, slot i -> 512*(i//16) + ref idx
        apgidx = np.zeros((P, 4), np.int16)
        for g in range(8):
            qg = 64 * g + gi
            iv = (V_REF * (gi // 16) + np.clip(gt_sh[qg, 2], 0, None)).astype(np.int16)
            apgidx[16 * g + gi % 16, gi // 16] = iv
        aux[:, 65:67] = apgidx.view(np.int32)

        probs_flat = np.concatenate(
            [
                rule_probs[lo:hi].reshape(-1),
                token_probs[lo:hi].reshape(-1),
                np.array([0.0, 1.0], np.float32),
            ]
        )
        ref_shuf = ref_sh[_QMAP.reshape(-1)].reshape(P, J * V_REF)
        in_maps.append(
            {
                "offs": offs_t,
                "aux": aux,
                "probs_flat": probs_flat.reshape(-1, 1),
                "ref_shuf": ref_shuf,
            }
        )
    return in_maps


def run(inputs, trace=False, trace_cores=None):
    """Run on the 8 NeuronCores; returns (scalar ndarray, BassKernelResults)."""
    from concourse.bass_utils import run_bass_kernel_spmd

    nc = get_nc()
    in_maps = make_in_maps(**inputs)
    res = run_bass_kernel_spmd(
        nc,
        in_maps,
        core_ids=list(range(N_CORES)),
        trace=trace,
        trace_cores=trace_cores,
    )
    total = np.float64(0.0)
    for r in res.results:
        total += np.float64(r["out"].sum(dtype=np.float64))
    return np.asarray(total, dtype=np.float32), res


def kernel(**inputs) -> np.ndarray:
    out, _ = run(inputs)
    return out
